# revision 9
# baseline (speedup 1.0000x reference)
"""Trainium2 Bass kernel for EquidistantDiscreteContinuousConv3d.

Math: out = conv3d(x, einsum('ogk,kzyx->ogzyx', weight, psi_local), stride 2,
pad 2) + bias, with x [2,8,128,128,128] -> out [2,16,64,64,64].

The dense 5^3 kernel only has taps within Euclidean radius 2 (33 of 125
offsets are nonzero). Sharding: 8 cores = batch(2) x z-groups(4); each core
computes out[b, :, 16g:16g+16] from an overlapping, zero-padded input slab.
No collectives — halos materialize as overlapping host-side slices.

Device mapping: the tensor engine contracts K = (z_local(16) x ic(8)) = 128
partitions, with M = (oz_sub(6) x oc(16)) = 96 packed into a block-banded
weight matrix (band encodes the 5 dz taps, zeros elsewhere), looped over the
13 (dy, dx) stencil taps that accumulate in PSUM. rhs slices come from a
phase-decomposed (even/odd y and x) view of the input tile so every tap is a
pure basic-slice access pattern at stride 2.
"""

import os

import ml_dtypes
import numpy as np

BF16 = ml_dtypes.bfloat16

IC, OC = 8, 16
TAPS_XY = [
    (dy, dx) for dy in range(-2, 3) for dx in range(-2, 3) if dy * dy + dx * dx <= 4
]  # 13 taps
OZ_PER = (6, 6, 4)
UNIT_FREE = 68 * 132  # y_unit x padded-x
N_CORES = 8

_MODULE = None
LAST_RESULT = None  # BassKernelResults of the most recent run (for test harness)


def _build_module():
    import concourse.bacc as bacc
    import concourse.mybir as mybir
    from concourse.tile import TileContext

    f32 = mybir.dt.float32
    bf16 = mybir.dt.bfloat16

    nc = bacc.Bacc()
    x_in = nc.dram_tensor("xc", [6, 128, UNIT_FREE], bf16, kind="ExternalInput")
    w_in = nc.dram_tensor("wc", [128, 13 * 96], bf16, kind="ExternalInput")
    out = nc.dram_tensor("out", [16, 16, 64, 64], f32, kind="ExternalOutput")

    with TileContext(nc) as tc:
        with (
            tc.tile_pool(name="wpool", bufs=1) as wpool,
            tc.tile_pool(name="xpool", bufs=3) as xpool,
            tc.tile_pool(name="spool", bufs=2) as spool,
            tc.tile_pool(name="ppool", bufs=4, space="PSUM") as ppool,
        ):
            wtile = wpool.tile([128, 13 * 96], bf16)
            nc.sync.dma_start(out=wtile[:], in_=w_in[:])
            for c in range(3):
                M = OZ_PER[c] * 16
                for h in range(2):
                    u = 2 * c + h
                    xt = xpool.tile([128, UNIT_FREE], bf16)
                    nc.sync.dma_start(out=xt[:], in_=x_in[u])
                    # free dims: (y_even 34, y_phase 2, x_even 66, x_phase 2)
                    x5 = xt.rearrange("p (a b c d) -> p a b c d", a=34, b=2, c=66, d=2)
                    stage = spool.tile([96, 4 * 512], f32)
                    for t in range(4):
                        ps = ppool.tile([96, 512], f32)
                        for j, (dy, dx) in enumerate(TAPS_XY):
                            jy, py = divmod(dy + 2, 2)
                            jx, px = divmod(dx + 2, 2)
                            rhs = x5[
                                :,
                                8 * t + jy : 8 * t + jy + 8,
                                py : py + 1,
                                jx : jx + 64,
                                px : px + 1,
                            ]
                            nc.tensor.matmul(
                                ps[:M],
                                wtile[:, j * 96 : j * 96 + M],
                                rhs,
                                start=(j == 0),
                                stop=(j == len(TAPS_XY) - 1),
                            )
                        nc.vector.tensor_copy(
                            out=stage[:M, t * 512 : (t + 1) * 512], in_=ps[:M]
                        )
                    dst = out[
                        6 * c : 6 * c + OZ_PER[c], :, 32 * h : 32 * h + 32, :
                    ].rearrange("a b c d -> (a b) (c d)")
                    nc.sync.dma_start(out=dst, in_=stage[:M])
    nc.compile()
    return nc


def _get_module():
    global _MODULE
    if _MODULE is None:
        _MODULE = _build_module()
    return _MODULE


def _band_weights(w5):
    """wc[k=(z*8+ic), j*96 + ozs*16 + oc] block-banded weight matrix."""
    wc = np.zeros((128, 13, 6, 16), np.float32)
    for j, (dy, dx) in enumerate(TAPS_XY):
        for dzi in range(5):
            dz = dzi - 2
            if dz * dz + dy * dy + dx * dx > 4:
                continue
            blk = w5[:, :, dzi, dy + 2, dx + 2].T  # [ic, oc]
            for ozs in range(6):
                z = 2 * ozs + dzi
                wc[z * 8 : (z + 1) * 8, j, ozs, :] = blk
    return np.ascontiguousarray(wc.reshape(128, 13 * 96))


def _shard_core_input(x, b, gz):
    """Per-core padded, chunked input: [6 units, 128 partitions, 68*132]."""
    xp = np.zeros((IC, 40, 132, 132), BF16)
    z_lo = 32 * gz - 2
    src_lo, src_hi = max(0, z_lo), min(128, z_lo + 40)
    xp[:, src_lo - z_lo : src_hi - z_lo, 2:130, 2:130] = x[b, :, src_lo:src_hi, :, :]
    units = np.empty((6, 128, UNIT_FREE), BF16)
    for c in range(3):
        for h in range(2):
            u = xp[:, 12 * c : 12 * c + 16, 64 * h : 64 * h + 68, :]
            units[2 * c + h] = u.transpose(1, 0, 2, 3).reshape(128, UNIT_FREE)
    return units


def kernel(x, weight, bias, psi_local):
    global LAST_RESULT
    from concourse.bass_utils import run_bass_kernel_spmd

    x = np.asarray(x, np.float32)
    weight = np.asarray(weight, np.float32)
    bias = np.asarray(bias, np.float32)
    psi_local = np.asarray(psi_local, np.float32)

    w5 = np.einsum("ogk,kzyx->ogzyx", weight, psi_local).astype(np.float32)
    wc = _band_weights(w5).astype(BF16)

    in_maps = []
    for core in range(N_CORES):
        b, gz = divmod(core, 4)
        in_maps.append({"xc": _shard_core_input(x, b, gz), "wc": wc})

    nc = _get_module()
    trace = bool(int(os.environ.get("KERNEL_TRACE", "0")))
    res = run_bass_kernel_spmd(
        nc, in_maps, core_ids=list(range(N_CORES)), trace=trace
    )
    LAST_RESULT = res

    out = np.empty((2, OC, 64, 64, 64), np.float32)
    for core in range(N_CORES):
        b, gz = divmod(core, 4)
        out[b, :, 16 * gz : 16 * gz + 16] = res.results[core]["out"].transpose(
            1, 0, 2, 3
        )
    out += bias[None, :, None, None, None]
    return out
